# Initial kernel scaffold
#
"""Trainium2 Bass kernel for a 3-layer stacked GRU + dual masked-linear heads.

Model (PyTorch GRUCell semantics, eval mode):
    h1,h2,h3 : 3 chained GRUCell layers over T=512 steps (B=32, F_IN=513, H=512)
    s1 = relu(h3_seq @ W_l1.T + b_l1); s2 = relu(h3_seq @ W_l2.T + b_l2)
    m1 = s1/(s1+s2+1e-16); m2 = s2/(s1+s2+1e-16)
    returns (m1*x, m2*x)

Device strategy:
  - "L-layout": per-step tensors as [128, 384] tiles; partition p=32q+b
    (q = H quarter, b = batch), free = 128g+j (g = gate r/z/n, j = H offset).
  - Matmuls keep h^T stationary, stream weights (float32r, N=384 >= 256 so
    full rate) through 4 concurrently-running column groups
    (tile_position=(0,32q)).
  - 3-layer wavefront: superstep s runs layer l at t=s-l+1; layer l>=2's
    input matmul consumes layer l-1's transposed h produced last superstep.
  - Biases enter via K=1 ones-row matmul rounds (b_hh -> P1 psum,
    b_ih -> P2 psum); layer-1's b_ih rides in the padded x k-tile (K=2).
  - The whole recurrence is replicated on every core (it is weight-streaming
    bound, so batch sharding gains nothing and per-step collectives are far
    too slow); the output phase is sharded over cores by time chunks.
"""

import os
import numpy as np

B, T, F, H = 32, 512, 513, 512
NCORES = 8

_CACHE = {}


# ---------------------------------------------------------------------------
# Host-side weight/input repacking (pure layout, no math beyond bias folding)
# ---------------------------------------------------------------------------

def _moving(W):
    """W [3H, K] (K multiple of 128) -> [128, KT, 4, 384] moving-operand tiles.

    out[kk, kt, q, 128*g+j] = W[g*512 + 128*q + j, 128*kt + kk]
    """
    K = W.shape[1]
    KT = K // 128
    Wk = W.reshape(3, 4, 128, KT, 128)  # [g, q, j, kt, kk]
    return np.ascontiguousarray(np.transpose(Wk, (4, 3, 1, 0, 2)).reshape(128, KT, 4, 384).astype(np.float16))


def _gate_rows(v):
    """v [3H] -> [4, 384] in (q, 128g+j) order."""
    return np.ascontiguousarray(np.transpose(v.reshape(3, 4, 128), (1, 0, 2)).reshape(4, 384).astype(np.float32))


def prep_inputs(inputs, t_steps):
    x = np.asarray(inputs["x"], np.float32)
    t = t_steps
    p = {}

    # Recurrent (h -> gates) weights, 3 layers stacked: [128, 3, 4, 4, 384]
    p["Whm"] = np.ascontiguousarray(np.stack(
        [_moving(np.asarray(inputs[f"W_hh{l}"], np.float32)) for l in (1, 2, 3)], axis=1))
    # Input (h_prev -> gates) weights for layers 2,3: [128, 2, 4, 4, 384]
    p["Wim"] = np.ascontiguousarray(np.stack(
        [_moving(np.asarray(inputs[f"W_ih{l}"], np.float32)) for l in (2, 3)], axis=1))

    # Layer-1 x weights: [128, 5, 4, 384]; k-tile 4 packs [W[:,512]; b_ih1]
    W1 = np.asarray(inputs["W_ih1"], np.float32)
    Wxm = np.zeros((128, 5, 4, 384), np.float16)
    Wxm[:, :4] = _moving(W1[:, :512])
    Wxm[0, 4] = _gate_rows(W1[:, 512])
    Wxm[1, 4] = _gate_rows(np.asarray(inputs["b_ih1"], np.float32))
    p["Wxm"] = Wxm

    # Bias rows, 32-partition-aligned so PE can use them as K=1 moving operands:
    # [128, 2, 1536]: (32r, 0, :) = row r for r in (b_hh1, b_hh2, b_hh3, b_ih2);
    # (0, 1, :) = b_ih3. Each row is the flattened [4, 384] gate layout.
    rows = ([_gate_rows(np.asarray(inputs[f"b_hh{l}"], np.float32)) for l in (1, 2, 3)]
            + [_gate_rows(np.asarray(inputs[f"b_ih{l}"], np.float32)) for l in (2, 3)])
    bias = np.zeros((128, 2, 1536), np.float16)
    for r in range(4):
        bias[32 * r, 0] = rows[r].reshape(-1)
    bias[0, 1] = rows[4].reshape(-1)
    p["bias"] = bias

    # Pre-transposed x stream tiles [T, 128, 5, 32]:
    #   [t, kk, kt, b] = x[b, t, 128*kt+kk] (kt<4); kt=4: row0 = x[b,t,512], row1 = 1.0
    xT = np.zeros((t, 128, 5, 32), np.float16)
    xT[:, :, :4] = np.transpose(x[:, :t, :512].reshape(B, t, 4, 128), (1, 3, 2, 0))
    xT[:, 0, 4] = x[:, :t, 512].T
    xT[:, 1, 4] = 1.0
    p["xT"] = np.ascontiguousarray(xT)

    # Output head weights [128, 2, 4, 513]: [kk, head, kt, f] = W_l[f, 128kt+kk]
    p["WlT"] = np.ascontiguousarray(np.stack(
        [np.transpose(np.asarray(inputs[f"W_l{i}"], np.float32).reshape(513, 4, 128), (2, 1, 0))
         for i in (1, 2)], axis=1).astype(np.float16))
    # Head biases [128, 2, 5]: [pp, head, m] = b_l[m*128+pp]  (padded to 640)
    bl = np.zeros((128, 2, 5), np.float32)
    for i in (1, 2):
        bp = np.zeros(640, np.float32)
        bp[:513] = np.asarray(inputs[f"b_l{i}"], np.float32)
        bl[:, i - 1, :] = bp.reshape(5, 128).T
    p["bl"] = bl

    # x for the output masking, f-major [5, 128, T, B] (padded f to 640)
    xo = np.zeros((5, 128, t, B), np.float32)
    xo.reshape(640, t, B)[:513] = np.transpose(x[:, :t, :], (2, 1, 0))
    p["xo"] = xo
    return p


# ---------------------------------------------------------------------------
# Device kernel
# ---------------------------------------------------------------------------

def build_nc(t_steps, shard_output):
    from contextlib import ExitStack
    import concourse.bacc as bacc
    import concourse.mybir as mybir
    import concourse.tile as tile
    from concourse.masks import make_identity

    f32 = mybir.dt.float32
    f16 = mybir.dt.float16
    AF = mybir.ActivationFunctionType
    ALU = mybir.AluOpType

    t_total = t_steps
    nc = bacc.Bacc("TRN2", target_bir_lowering=False)

    # ---- DRAM I/O -------------------------------------------------------
    xT_d = nc.dram_tensor("xT", [t_total, 128, 5, 32], f16, kind="ExternalInput")
    Whm_d = nc.dram_tensor("Whm", [128, 3, 4, 4, 384], f16, kind="ExternalInput")
    Wim_d = nc.dram_tensor("Wim", [128, 2, 4, 4, 384], f16, kind="ExternalInput")
    Wxm_d = nc.dram_tensor("Wxm", [128, 5, 4, 384], f16, kind="ExternalInput")
    bias_d = nc.dram_tensor("bias", [128, 2, 1536], f16, kind="ExternalInput")
    WlT_d = nc.dram_tensor("WlT", [128, 2, 4, 513], f16, kind="ExternalInput")
    bl_d = nc.dram_tensor("bl", [128, 2, 5], f32, kind="ExternalInput")
    xo_d = nc.dram_tensor("xo", [5, 128, t_total, B], f32, kind="ExternalInput")
    # outputs in f-major layout [5, 128, T, B] (host transposes to [B, T, F])
    out1_d = nc.dram_tensor("out1", [5, 128, t_total, B], f32, kind="ExternalOutput")
    out2_d = nc.dram_tensor("out2", [5, 128, t_total, B], f32, kind="ExternalOutput")

    with ExitStack() as ctx:
        tc = ctx.enter_context(tile.TileContext(nc))

        consts = ctx.enter_context(tc.tile_pool(name="consts", bufs=1))
        ident = consts.tile([128, 128], f32)
        make_identity(nc, ident)
        ones = consts.tile([128, 128], f16)
        nc.vector.memset(ones, 1.0)

        # DRAM scratch for the h3 transpose stream (consumed by output phase)
        dram = ctx.enter_context(tc.tile_pool(name="dram", bufs=1, space="DRAM"))
        h3T = dram.tile([t_total, 128, 128], f16)

        with ExitStack() as rctx:
            wrec = rctx.enter_context(tc.tile_pool(name="wrec", bufs=1))
            Whm = wrec.tile([128, 3, 4, 4, 384], f16)
            nc.sync.dma_start(out=Whm, in_=Whm_d[:, :, :, :, :])
            Wim = wrec.tile([128, 2, 4, 4, 384], f16)
            nc.sync.dma_start(out=Wim, in_=Wim_d[:, :, :, :, :])
            Wxm = wrec.tile([128, 5, 4, 384], f16)
            nc.sync.dma_start(out=Wxm, in_=Wxm_d[:, :, :, :])
            bias_sb = wrec.tile([128, 2, 1536], f16)
            nc.sync.dma_start(out=bias_sb, in_=bias_d[:, :, :])

            xpool = rctx.enter_context(tc.tile_pool(name="xpool", bufs=4))
            hpool = rctx.enter_context(tc.tile_pool(name="hpool", bufs=2))
            gpool = rctx.enter_context(tc.tile_pool(name="gpool", bufs=2))
            p1p = rctx.enter_context(tc.tile_pool(name="p1p", bufs=1, space="PSUM"))
            p2p = rctx.enter_context(tc.tile_pool(name="p2p", bufs=1, space="PSUM"))
            tpp = rctx.enter_context(tc.tile_pool(name="tpp", bufs=2, space="PSUM"))

            def emit_layer(l, t, src_hT, own_hT, own_hL):
                """One GRU cell step for layer l at time t. Returns (hT_new, hL_new)."""
                li = l - 1
                P1 = p1p.tile([128, 384], f32, tag=f"p1_{l}")
                P2 = p2p.tile([128, 384], f32, tag=f"p2_{l}")

                def bias_mm(o, row, q, start, stop):
                    rp, blk = (32 * row, 0) if row < 4 else (0, 1)
                    nc.tensor.matmul(
                        o, ones[rp:rp + 1, 32 * q:32 * q + 32],
                        bias_sb[rp:rp + 1, blk, 384 * q:384 * q + 384],
                        start=start, stop=stop, tile_position=(rp, 32 * q))

                # ---- P2: input-side pre-activations (gi + b_ih) ----
                if l == 1:
                    xt = xpool.tile([128, 5, 32], f16)
                    nc.sync.dma_start(out=xt, in_=xT_d[t, :, :, :])
                    for q in range(4):
                        o = P2[32 * q:32 * q + 32, :]
                        for kt in range(5):
                            kk = 128 if kt < 4 else 2
                            nc.tensor.matmul(
                                o, xt[:kk, kt, :], Wxm[:kk, kt, q, :],
                                start=(kt == 0), stop=(kt == 4),
                                tile_position=(0, 32 * q))
                else:
                    for q in range(4):
                        o = P2[32 * q:32 * q + 32, :]
                        bias_mm(o, li + 2, q, True, False)
                        for kt in range(4):
                            nc.tensor.matmul(
                                o, src_hT[:, 32 * kt:32 * kt + 32], Wim[:, li - 1, kt, q, :],
                                start=False, stop=(kt == 3),
                                tile_position=(0, 32 * q))

                # ---- P1: recurrent-side pre-activations (gh + b_hh) ----
                for q in range(4):
                    o = P1[32 * q:32 * q + 32, :]
                    bias_mm(o, li, q, True, t == 0)
                    if t > 0:
                        for kt in range(4):
                            nc.tensor.matmul(
                                o, own_hT[:, 32 * kt:32 * kt + 32], Whm[:, li, kt, q, :],
                                start=False, stop=(kt == 3),
                                tile_position=(0, 32 * q))

                # ---- gates ----
                # a TensorTensor may read at most one PSUM operand; stage P2's
                # r|z half through SBUF (it is off the critical gh chain)
                g2 = gpool.tile([128, 256], f32, tag=f"dd_{l}")
                nc.scalar.copy(g2, P2[:, 0:256])
                rz = gpool.tile([128, 256], f32, tag=f"ca_{l}")
                nc.vector.tensor_add(rz, P1[:, 0:256], g2)
                nc.scalar.activation(rz, rz, AF.Sigmoid)
                r = rz[:, 0:128]
                z = rz[:, 128:256]

                rn = gpool.tile([128, 128], f32, tag=f"aa_{l}")
                nc.vector.tensor_mul(rn, r, P1[:, 256:384])
                n = gpool.tile([128, 128], f32, tag=f"bb_{l}")
                nc.vector.tensor_add(n, rn, P2[:, 256:384])
                nc.scalar.activation(n, n, AF.Tanh)

                w = gpool.tile([128, 128], f32, tag=f"aa_{l}")
                nc.vector.tensor_scalar(w, z, -1.0, 1.0, ALU.mult, ALU.add)
                zh = gpool.tile([128, 128], f32, tag=f"bb_{l}")
                if t > 0:
                    nc.vector.tensor_mul(zh, z, own_hL)
                else:
                    nc.vector.memset(zh, 0.0)
                wn = gpool.tile([128, 128], f32, tag=f"ca_{l}")
                nc.vector.tensor_mul(wn, w, n)
                hL_new = hpool.tile([128, 128], f32, tag=f"hL_{l}")
                nc.vector.tensor_add(hL_new, wn, zh)

                tp = tpp.tile([128, 128], f32, tag="tp")
                nc.tensor.transpose(tp, hL_new, ident)
                hT_new = hpool.tile([128, 128], f16, tag=f"hT_{l}")
                nc.scalar.copy(hT_new, tp)

                if l == 3:
                    nc.sync.dma_start(out=h3T[t, :, :], in_=hT_new)
                return hT_new, hL_new

            hT_cur = {1: None, 2: None, 3: None}
            hL_cur = {1: None, 2: None, 3: None}
            for s in range(t_total + 2):
                cons = {2: hT_cur[1], 3: hT_cur[2]}
                for l in (1, 2, 3):
                    t = s - (l - 1)
                    if 0 <= t < t_total:
                        hT_cur[l], hL_cur[l] = emit_layer(
                            l, t, cons.get(l), hT_cur[l], hL_cur[l])

        # ---- output phase: s1/s2 heads + masking ------------------------
        with ExitStack() as octx:
            wout = octx.enter_context(tc.tile_pool(name="wout", bufs=1))
            WlT = wout.tile([128, 2, 4, 513], f16)
            nc.sync.dma_start(out=WlT, in_=WlT_d[:, :, :, :])
            bl = wout.tile([128, 2, 5], f32)
            nc.sync.dma_start(out=bl, in_=bl_d[:, :, :])

            opool = octx.enter_context(tc.tile_pool(name="opool", bufs=3))
            spool = octx.enter_context(tc.tile_pool(name="spool", bufs=2))
            opsum = octx.enter_context(tc.tile_pool(name="opsum", bufs=4, space="PSUM"))

            tc_chunk = min(16, t_total)  # timesteps per chunk -> N = 16*32 = 512
            assert t_total % tc_chunk == 0
            nchunks = t_total // tc_chunk
            for c in range(nchunks):
                t0 = c * tc_chunk
                rhs = []
                for kt in range(4):
                    rt = opool.tile([128, tc_chunk * 32], f16, tag=f"rhs{kt}")
                    nc.sync.dma_start(
                        out=rt,
                        in_=h3T[t0:t0 + tc_chunk, :, 32 * kt:32 * kt + 32]
                        .rearrange("t k b -> k t b"))
                    rhs.append(rt)
                for m in range(5):
                    fp = 128 if m < 4 else 1
                    xt = opool.tile([128, tc_chunk * 32], f32, tag="xchunk")
                    nc.sync.dma_start(
                        out=xt[:fp], in_=xo_d[m, 0:fp, t0:t0 + tc_chunk, :])
                    ss = []
                    for hd in range(2):
                        ps = opsum.tile([128, tc_chunk * 32], f32, tag=f"ops{hd}")
                        for kt in range(4):
                            nc.tensor.matmul(
                                ps[:fp], WlT[:, hd, kt, m * 128:m * 128 + fp],
                                rhs[kt], start=(kt == 0), stop=(kt == 3))
                        s = spool.tile([128, tc_chunk * 32], f32, tag=f"s{hd}")
                        nc.scalar.activation(
                            s[:fp], ps[:fp], AF.Relu, bias=bl[0:fp, hd, m:m + 1])
                        ss.append(s)
                    den = spool.tile([128, tc_chunk * 32], f32, tag="den")
                    nc.vector.tensor_add(den[:fp], ss[0][:fp], ss[1][:fp])
                    nc.vector.tensor_scalar_add(den[:fp], den[:fp], 1e-16)
                    rden = spool.tile([128, tc_chunk * 32], f32, tag="rden")
                    nc.vector.reciprocal(rden[:fp], den[:fp])
                    xr = spool.tile([128, tc_chunk * 32], f32, tag="xr")
                    nc.vector.tensor_mul(xr[:fp], xt[:fp], rden[:fp])
                    for hd, out_d in ((0, out1_d), (1, out2_d)):
                        o = spool.tile([128, tc_chunk * 32], f32, tag=f"o{hd}")
                        nc.vector.tensor_mul(o[:fp], ss[hd][:fp], xr[:fp])
                        nc.sync.dma_start(
                            out=out_d[m, 0:fp, t0:t0 + tc_chunk, :],
                            in_=o[:fp].rearrange("f (t b) -> f t b", b=32))

    nc.finalize()
    return nc


# ---------------------------------------------------------------------------
# Entry point
# ---------------------------------------------------------------------------

class _Runner:
    """Caches the compiled PJRT executable so repeat calls only pay
    dispatch + device execution (mirrors bass2jax.run_bass_via_pjrt)."""

    def __init__(self, nc, n_cores):
        import jax
        import concourse.mybir as mybir
        from concourse import bass2jax
        from concourse.bass2jax import (
            _bass_exec_p, install_neuronx_cc_hook, partition_id_tensor)
        from jax.experimental.shard_map import shard_map
        from jax.sharding import Mesh, PartitionSpec

        install_neuronx_cc_hook()
        self.jax = jax
        self.n_cores = n_cores
        partition_name = (nc.partition_id_tensor.name
                          if nc.partition_id_tensor else None)
        in_names, out_names, out_avals, zero_outs = [], [], [], []
        for alloc in nc.m.functions[0].allocations:
            if not isinstance(alloc, mybir.MemoryLocationSet):
                continue
            name = alloc.memorylocations[0].name
            if alloc.kind == "ExternalInput":
                if name != partition_name:
                    in_names.append(name)
            elif alloc.kind == "ExternalOutput":
                shape = tuple(alloc.tensor_shape)
                dtype = mybir.dt.np(alloc.dtype)
                out_names.append(name)
                out_avals.append(jax.core.ShapedArray(shape, dtype))
                zero_outs.append(np.zeros(shape, dtype))
        n_params = len(in_names)
        self.in_names = list(in_names)
        self.out_names = out_names
        self.out_avals = out_avals
        self.zero_outs = zero_outs
        all_in = in_names + out_names
        if partition_name is not None:
            all_in.append(partition_name)

        def _body(*args):
            operands = list(args)
            if partition_name is not None:
                operands.append(partition_id_tensor())
            return tuple(_bass_exec_p.bind(
                *operands, out_avals=tuple(out_avals), in_names=tuple(all_in),
                out_names=tuple(out_names), lowering_input_output_aliases=(),
                sim_require_finite=True, sim_require_nnan=True, nc=nc))

        devices = jax.devices()[:n_cores]
        self.mesh = Mesh(np.asarray(devices), ("core",))
        self.pspec = PartitionSpec("core")
        n_out = len(out_names)
        self.sharded = jax.jit(
            shard_map(_body, mesh=self.mesh,
                      in_specs=(self.pspec,) * (n_params + n_out),
                      out_specs=(self.pspec,) * n_out,
                      check_rep=False),
            keep_unused=True)

    def prepare(self, in_map):
        """Concat per-core inputs + zero out-buffers, device_put once."""
        import jax
        from jax.sharding import NamedSharding
        sh = NamedSharding(self.mesh, self.pspec)
        args = [np.concatenate([np.asarray(in_map[n])] * self.n_cores, axis=0)
                for n in self.in_names]
        args += [np.zeros((self.n_cores * z.shape[0], *z.shape[1:]), z.dtype)
                 for z in self.zero_outs]
        return [jax.device_put(a, sh) for a in args]

    def call(self, concat_in):
        return self.sharded(*concat_in)

    def results0(self, outs):
        """Core-0 slice of each output, as numpy."""
        res = {}
        for i, name in enumerate(self.out_names):
            a = np.asarray(outs[i])
            res[name] = a.reshape(self.n_cores, *self.out_avals[i].shape)[0]
        return res


def _get_runner(t_steps):
    key = (t_steps, False)
    if key not in _CACHE:
        nc = build_nc(t_steps, False)
        _CACHE[key] = _Runner(nc, NCORES)
    return _CACHE[key]


def _run(inputs, t_steps=T, trace=False, time_reps=0):
    import time as _time
    r = _get_runner(t_steps)
    p = prep_inputs(inputs, t_steps)
    concat_in = r.prepare(p)
    outs = r.call(concat_in)  # first call compiles
    out = r.results0(outs)
    o1 = _unpack_out(out["out1"], t_steps)
    o2 = _unpack_out(out["out2"], t_steps)

    times = []
    for _ in range(time_reps):
        t0 = _time.time()
        outs = r.call(concat_in)
        for o in outs:
            o.block_until_ready()
        times.append(_time.time() - t0)
    return (o1, o2), times


def _unpack_out(o, t_steps):
    """[5, 128, T, B] f-major -> [B, T, 513]."""
    return np.ascontiguousarray(
        np.transpose(o.reshape(640, t_steps, B)[:F], (2, 1, 0)))


def kernel(**inputs):
    (o1, o2), _ = _run(inputs, T)
    return (o1, o2)



# revision 11
# speedup vs baseline: 180.7003x; 180.7003x over previous
"""Trainium2 Bass kernel for a 3-layer stacked GRU + dual masked-linear heads.

Model (PyTorch GRUCell semantics, eval mode):
    h1,h2,h3 : 3 chained GRUCell layers over T=512 steps (B=32, F_IN=513, H=512)
    s1 = relu(h3_seq @ W_l1.T + b_l1); s2 = relu(h3_seq @ W_l2.T + b_l2)
    m1 = s1/(s1+s2+1e-16); m2 = s2/(s1+s2+1e-16)
    returns (m1*x, m2*x)

Device strategy (v2):
  - Cell layout: gate tensors [128p = 32q+b, 384f = 128g+j] (q = H quarter,
    b = batch, g = gate r/z/n, j = H offset); hidden h = 128q+j.
  - Per-step PE work is ONLY the recurrent matmul gh = h@W_hh.T: 16 MMs
    (4 k-tiles x 4 col strips, kt-outer emission). The input-side matmuls
    (gi) for every layer are batched 4 timesteps at a time into full-M=128
    GEMMs (stationary = transposed source [128k, (4t,32b)]), with biases
    folded in as K=1 ones-row rounds; results roundtrip through DRAM to
    convert from (t,b)-partition layout back into cell layout.
  - 3-layer wavefront at block (4-step) granularity, layer l lagging 2(l-1)
    blocks; per block-superstep each layer runs 1 GEMM + 4 cells. PE stream
    stays stall-free: cell MMs, GEMM rounds, and transposes are emitted so
    independent groups are always in flight (the PE overlaps M=32 strip
    matmuls ~4x when independent work is available).
  - Gate math spread over DVE/ACT/GpSimd: rz=sigmoid(gh_rz+gi_rz),
    n=tanh(gi_n + r*gh_n), h' = n + z*(h-n).
  - The recurrence is replicated on all 8 cores (collective latency makes
    per-step communication infeasible); the output phase (2 linear heads +
    soft masks) is sharded over cores by time via the partition id.
"""

import numpy as np

B, T, F, H = 32, 512, 513, 512
NCORES = 8

_CACHE = {}


# ---------------------------------------------------------------------------
# Host-side weight/input repacking (pure layout, no math beyond bias folding)
# ---------------------------------------------------------------------------

def _gmov(W):
    """W [3H, K] (K mult of 128) -> [128, KT, 1536] moving tiles.

    out[kk, kt, 384q+128g+j] = W[512g + 128q + j, 128kt + kk]
    """
    K = W.shape[1]
    KT = K // 128
    Wk = W.reshape(3, 4, 128, KT, 128)  # [g, q, j, kt, kk]
    return np.ascontiguousarray(
        np.transpose(Wk, (4, 3, 1, 0, 2)).reshape(128, KT, 1536).astype(np.float16))


def _gate_row(v):
    """v [3H] -> [1536] in (q, g, j) order."""
    return np.ascontiguousarray(
        np.transpose(v.reshape(3, 4, 128), (1, 0, 2)).reshape(1536).astype(np.float16))


def prep_inputs(inputs, t_steps):
    x = np.asarray(inputs["x"], np.float32)
    t = t_steps
    G = t // 4
    p = {}

    # gh (recurrent) weights, 3 layers: [128, 3, 4, 1536]
    p["Whm"] = np.ascontiguousarray(np.stack(
        [_gmov(np.asarray(inputs[f"W_hh{l}"], np.float32)) for l in (1, 2, 3)],
        axis=1))

    # gi GEMM weights for layers 2,3: [128, 2, 4, 1536]
    p["Wim"] = np.ascontiguousarray(np.stack(
        [_gmov(np.asarray(inputs[f"W_ih{l}"], np.float32)) for l in (2, 3)],
        axis=1))

    # Gi biases as K=1 rows: [128, 2, 1536]. PyTorch GRU: the r/z gates can
    # take b_ih+b_hh combined, but b_hh_n must stay INSIDE the r-multiply
    # (n = tanh(i_n + b_ih_n + r*(h_n + b_hh_n))), so Gi gets b_ih_n only.
    def gi_bias(l):
        bi = np.asarray(inputs[f"b_ih{l}"], np.float32)
        bh = np.asarray(inputs[f"b_hh{l}"], np.float32).copy()
        bh[2 * H:] = 0.0  # drop b_hh_n; it rides with gh instead
        return _gate_row(bi + bh)

    b23 = np.zeros((128, 2, 1536), np.float16)
    for i, l in enumerate((2, 3)):
        b23[0, i] = gi_bias(l)
    p["b23"] = b23

    # b_hh_n rows for the gh-side bias matmul: [128, 3, 512]
    #   [0, l, 128q+j] = b_hh_{l+1}[2H + 128q + j]
    bhn = np.zeros((128, 3, 512), np.float16)
    for i, l in enumerate((1, 2, 3)):
        bhn[0, i] = np.asarray(inputs[f"b_hh{l}"], np.float32)[2 * H:]
    p["bhn"] = bhn

    # Layer-1 x weights [128, 5, 1536]; k-tile 4 rows: [W[:,512]; gi bias]
    W1 = np.asarray(inputs["W_ih1"], np.float32)
    Wx = np.zeros((128, 5, 1536), np.float16)
    Wx[:, :4] = _gmov(W1[:, :512])
    Wx[0, 4] = _gate_row(W1[:, 512])
    Wx[1, 4] = gi_bias(1)
    p["Wx"] = Wx

    # Pre-transposed x for the Gi1 GEMM: [G, 128, 5, 128]
    #   [g, kk, kt, 32*ti+b] = x[b, 4g+ti, 128kt+kk]  (kt<4)
    #   kt=4: row0 = x[b, t, 512], row1 = 1.0
    xT4 = np.zeros((G, 128, 5, 128), np.float16)
    xr = x[:, :t, :512].reshape(B, G, 4, 4, 128)        # [b, g, ti, kt, kk]
    xT4[:, :, :4, :] = np.transpose(xr, (1, 4, 3, 2, 0)).reshape(G, 128, 4, 128)
    xT4[:, 0, 4, :] = np.transpose(
        x[:, :t, 512].reshape(B, G, 4), (1, 2, 0)).reshape(G, 128)
    xT4[:, 1, 4, :] = 1.0
    p["xT4"] = np.ascontiguousarray(xT4)

    # Output head weights [128, 2, 4, 513]: [kk, head, kt, f] = W_l[f, 128kt+kk]
    p["WlT"] = np.ascontiguousarray(np.stack(
        [np.transpose(np.asarray(inputs[f"W_l{i}"], np.float32).reshape(513, 4, 128),
                      (2, 1, 0)) for i in (1, 2)], axis=1).astype(np.float16))
    # Head biases [128, 2, 5]: [pp, head, m] = b_l[m*128+pp]  (padded to 640)
    bl = np.zeros((128, 2, 5), np.float32)
    for i in (1, 2):
        bp = np.zeros(640, np.float32)
        bp[:513] = np.asarray(inputs[f"b_l{i}"], np.float32)
        bl[:, i - 1, :] = bp.reshape(5, 128).T
    p["bl"] = bl

    # x for the output masking, f-major [5, 128, T, B] f16 (padded f to 640)
    xo = np.zeros((5, 128, t, B), np.float16)
    xo.reshape(640, t, B)[:513] = np.transpose(x[:, :t, :], (2, 1, 0))
    p["xo"] = xo
    return p


# ---------------------------------------------------------------------------
# Device kernel
# ---------------------------------------------------------------------------

def build_nc(t_steps, shard_output):
    from contextlib import ExitStack
    import concourse.bacc as bacc
    import concourse.mybir as mybir
    import concourse.tile as tile
    import concourse.bass as bass
    from concourse.masks import make_identity

    f32 = mybir.dt.float32
    f16 = mybir.dt.float16
    AF = mybir.ActivationFunctionType
    ALU = mybir.AluOpType

    t_total = t_steps
    G = t_total // 4
    assert t_total % 4 == 0
    nc = bacc.Bacc("TRN2", target_bir_lowering=False, num_devices=NCORES)

    # ---- DRAM I/O -------------------------------------------------------
    xT4_d = nc.dram_tensor("xT4", [G, 128, 5, 128], f16, kind="ExternalInput")
    Whm_d = nc.dram_tensor("Whm", [128, 3, 4, 1536], f16, kind="ExternalInput")
    Wim_d = nc.dram_tensor("Wim", [128, 2, 4, 1536], f16, kind="ExternalInput")
    Wx_d = nc.dram_tensor("Wx", [128, 5, 1536], f16, kind="ExternalInput")
    b23_d = nc.dram_tensor("b23", [128, 2, 1536], f16, kind="ExternalInput")
    bhn_d = nc.dram_tensor("bhn", [128, 3, 512], f16, kind="ExternalInput")
    WlT_d = nc.dram_tensor("WlT", [128, 2, 4, 513], f16, kind="ExternalInput")
    bl_d = nc.dram_tensor("bl", [128, 2, 5], f32, kind="ExternalInput")
    xo_d = nc.dram_tensor("xo", [5, 128, t_total, B], f16, kind="ExternalInput")
    # outputs in f-major layout [5, 128, T, B] (host transposes to [B, T, F])
    out1_d = nc.dram_tensor("out1", [5, 128, t_total, B], f32, kind="ExternalOutput")
    out2_d = nc.dram_tensor("out2", [5, 128, t_total, B], f32, kind="ExternalOutput")

    with ExitStack() as ctx:
        tc = ctx.enter_context(tile.TileContext(nc))

        consts = ctx.enter_context(tc.tile_pool(name="consts", bufs=1))
        ident = consts.tile([128, 128], f32)
        make_identity(nc, ident)
        ones = consts.tile([128, 128], f16)
        nc.vector.memset(ones, 1.0)

        # DRAM scratch
        dram = ctx.enter_context(tc.tile_pool(name="dram", bufs=1, space="DRAM"))
        h3T = dram.tile([t_total, 128, 128], f16)
        # Gi layout: [block, ti, 32q+b, 384(g,j)] so per-step readback is 2D
        Gi_d = [dram.tile([G, 4, 128, 384], f16, name=f"gi{l}", tag=f"gi{l}")
                for l in range(3)]

        with ExitStack() as rctx:
            wrec = rctx.enter_context(tc.tile_pool(name="wrec", bufs=1))
            Whm = wrec.tile([128, 3, 4, 1536], f16)
            nc.sync.dma_start(out=Whm, in_=Whm_d[:, :, :, :])
            Wim = wrec.tile([128, 2, 4, 1536], f16)
            nc.sync.dma_start(out=Wim, in_=Wim_d[:, :, :, :])
            Wx = wrec.tile([128, 5, 1536], f16)
            nc.sync.dma_start(out=Wx, in_=Wx_d[:, :, :])
            b23 = wrec.tile([128, 2, 1536], f16)
            nc.sync.dma_start(out=b23, in_=b23_d[:, :, :])
            bhn = wrec.tile([128, 3, 512], f16)
            nc.sync.dma_start(out=bhn, in_=bhn_d[:, :, :])

            xt4p = rctx.enter_context(tc.tile_pool(name="xt4p", bufs=3))
            ht4p = rctx.enter_context(tc.tile_pool(name="ht4p", bufs=3))
            gip = rctx.enter_context(tc.tile_pool(name="gip", bufs=9))
            gatep = rctx.enter_context(tc.tile_pool(name="gatep", bufs=2))
            hlp = rctx.enter_context(tc.tile_pool(name="hlp", bufs=2))
            stagep = rctx.enter_context(tc.tile_pool(name="stagep", bufs=6))
            cellps = rctx.enter_context(tc.tile_pool(name="cellps", bufs=1, space="PSUM"))
            gemmps = rctx.enter_context(tc.tile_pool(name="gemmps", bufs=2, space="PSUM"))
            tpps = rctx.enter_context(tc.tile_pool(name="tpps", bufs=3, space="PSUM"))

            # wavefront state (python references to live tiles)
            hT4_cur = {l: None for l in (1, 2, 3)}   # ring tile for block being written
            hT4_prev = {l: None for l in (1, 2, 3)}  # previous block's ring tile
            hL = {l: None for l in (1, 2, 3)}
            xt4_tiles = {}                            # g -> tile
            gi_tiles = {}                             # (l, t) -> tile

            def emit_xt4_fetch(g):
                if not (0 <= g < G):
                    return
                xt = xt4p.tile([128, 5, 128], f16, tag="xt4", name="xt4")
                nc.sync.dma_start(out=xt, in_=xT4_d[g, :, :, :])
                xt4_tiles[g] = xt

            def emit_gi_fetch(l, g):
                """Prefetch the 4 per-step Gi tiles of block g for layer l."""
                if not (0 <= g < G):
                    return
                for ti in range(4):
                    gi = gip.tile([128, 384], f16, tag=f"gi{l}", name="gi")
                    nc.sync.dma_start(out=gi, in_=Gi_d[l - 1][g, ti, :, :])
                    gi_tiles[(l, 4 * g + ti)] = gi

            def emit_gemm(l, g):
                """Gi GEMM for layer l, block g -> DRAM Gi_d[l-1][g]."""
                if not (0 <= g < G):
                    return
                if l == 1:
                    xt = xt4_tiles.pop(g)
                else:
                    src = hT4_blocks[(l - 1, g)]
                for s in range(4):
                    ps = gemmps.tile([128, 384], f32, tag="gps", name="gps")
                    if l == 1:
                        for kt in range(4):
                            nc.tensor.matmul(
                                ps, xt[:, kt, :], Wx[:, kt, 384 * s:384 * s + 384],
                                start=(kt == 0), stop=False)
                        nc.tensor.matmul(
                            ps, xt[0:2, 4, :], Wx[0:2, 4, 384 * s:384 * s + 384],
                            start=False, stop=True)
                    else:
                        for kt in range(4):
                            nc.tensor.matmul(
                                ps, src[:, kt, :, :],
                                Wim[:, l - 2, kt, 384 * s:384 * s + 384],
                                start=(kt == 0), stop=False)
                        nc.tensor.matmul(
                            ps, ones[0:1, 0:128],
                            b23[0:1, l - 2, 384 * s:384 * s + 384],
                            start=False, stop=True)
                    st = stagep.tile([128, 384], f16, tag="stg", name="stg")
                    if s % 2 == 1:
                        nc.scalar.copy(st, ps)
                    else:
                        nc.vector.tensor_copy(st, ps)
                    nc.sync.dma_start(
                        out=Gi_d[l - 1][g, :, 32 * s:32 * s + 32, :], in_=st)

            hT4_blocks = {}  # (l, g) -> ring tile (kept until consumed by GEMM)

            def emit_cell_mm(l, g, ti):
                """PE part of cell (l, t): gh accumulation into PSUM.

                P[:, 0:256] = (h @ W_hh.T)_{r|z}  (t>0; garbage at t=0, unread)
                P[:, 256:384] = (h @ W_hh.T)_n + b_hh_n
                """
                t = 4 * g + ti
                P = cellps.tile([128, 384], f32, tag=f"p{l}", name=f"p{l}")
                if t > 0:
                    hb = hT4_cur[l] if ti > 0 else hT4_prev[l]
                    tp_i = (ti - 1) % 4
                    for kt in range(4):
                        for q in range(4):
                            nc.tensor.matmul(
                                P[32 * q:32 * q + 32, :],
                                hb[:, kt, tp_i, :],
                                Whm[:, l - 1, kt, 384 * q:384 * q + 384],
                                start=(kt == 0), stop=False,
                                tile_position=(0, 32 * q))
                for q in range(4):
                    nc.tensor.matmul(
                        P[32 * q:32 * q + 32, 256:384],
                        ones[0:1, 32 * q:32 * q + 32],
                        bhn[0:1, l - 1, 128 * q:128 * q + 128],
                        start=(t == 0), stop=True,
                        tile_position=(0, 32 * q), skip_group_check=True)
                return P

            def emit_cell_gates(l, g, ti, P):
                """Vector/scalar gate math for cell (l, t) -> new hL."""
                t = 4 * g + ti
                gi = gi_tiles.pop((l, t))
                if t > 0:
                    rz = gatep.tile([128, 256], f32, tag=f"rz{l}", name="rz")
                    nc.vector.tensor_add(rz, P[:, 0:256], gi[:, 0:256])
                else:
                    rz = gi[:, 0:256]
                rzS = gatep.tile([128, 256], f32, tag=f"rzs{l}", name="rzS")
                nc.scalar.activation(rzS[:, 0:128], rz[:, 0:128], AF.Sigmoid)
                nc.scalar.activation(rzS[:, 128:256], rz[:, 128:256], AF.Sigmoid)
                rn = gatep.tile([128, 128], f32, tag=f"rn{l}", name="rn")
                nc.vector.tensor_mul(rn, rzS[:, 0:128], P[:, 256:384])
                nt = gatep.tile([128, 128], f32, tag=f"nt{l}", name="nt")
                nc.vector.tensor_add(nt, rn, gi[:, 256:384])
                nS = gatep.tile([128, 128], f32, tag=f"ns{l}", name="nS")
                nc.scalar.activation(nS, nt, AF.Tanh)
                d = gatep.tile([128, 128], f32, tag=f"d{l}", name="d")
                if t > 0:
                    nc.gpsimd.tensor_sub(d, hL[l], nS)
                else:
                    nc.gpsimd.tensor_scalar_mul(d, nS, -1.0)
                zd = gatep.tile([128, 128], f32, tag=f"zd{l}", name="zd")
                nc.gpsimd.tensor_mul(zd, rzS[:, 128:256], d)
                hL_new = hlp.tile([128, 128], f32, tag=f"hl{l}", name="hL")
                nc.vector.tensor_add(hL_new, zd, nS)
                hL[l] = hL_new
                return hL_new

            def emit_cell_tp(l, g, ti, hL_new):
                """Transpose h' and store into the hT4 ring (f16)."""
                t = 4 * g + ti
                tp = tpps.tile([128, 128], f32, tag="tp", name="tp")
                nc.tensor.transpose(tp, hL_new, ident)
                hb = hT4_cur[l]
                nc.vector.tensor_copy(hb[:, :, ti, :], tp)
                if l == 3:
                    nc.sync.dma_start(out=h3T[t, :, :], in_=hb[:, :, ti, :])

            # lags (in blocks) per layer
            LAG = {1: 0, 2: 2, 3: 4}
            NBS = G + LAG[3] + 1

            emit_xt4_fetch(0)
            emit_xt4_fetch(1)

            for bs in range(NBS):
                # prefetches for upcoming blocks
                emit_xt4_fetch(bs + 2)
                # GEMMs: Gi1 for block bs+1; Gi2 for block bs-1; Gi3 for bs-3
                emit_gemm(1, bs + 1)
                if bs == 0:
                    emit_gemm(1, 0)
                emit_gemm(2, bs - 1)
                emit_gemm(3, bs - 3)
                # Gi read prefetch (block that each layer runs NEXT superstep)
                emit_gi_fetch(1, bs + 1 if bs > 0 else 0)
                if bs == 0:
                    emit_gi_fetch(1, 1)
                emit_gi_fetch(2, bs + 1 - LAG[2])
                emit_gi_fetch(3, bs + 1 - LAG[3])

                # new ring tiles for blocks being produced this superstep
                for l in (1, 2, 3):
                    g = bs - LAG[l]
                    if 0 <= g < G:
                        hT4_prev[l] = hT4_cur[l]
                        hb = ht4p.tile([128, 4, 4, 32], f16, tag=f"ht4_{l}",
                                       name="hb")
                        hT4_cur[l] = hb
                        hT4_blocks[(l, g)] = hb

                # cells: ti-major across layers
                for ti in range(4):
                    Ps = {}
                    for l in (1, 2, 3):
                        g = bs - LAG[l]
                        if 0 <= g < G:
                            Ps[l] = emit_cell_mm(l, g, ti)
                    for l in (1, 2, 3):
                        g = bs - LAG[l]
                        if 0 <= g < G:
                            hn = emit_cell_gates(l, g, ti, Ps[l])
                            emit_cell_tp(l, g, ti, hn)

                # drop GEMM-consumed ring references
                for l in (2, 3):
                    hT4_blocks.pop((l - 1, bs - LAG[l] - 1 - (2 if l == 3 else 0)), None)

        # ---- output phase: s1/s2 heads + masking ------------------------
        with ExitStack() as octx:
            wout = octx.enter_context(tc.tile_pool(name="wout", bufs=1))
            WlT = wout.tile([128, 2, 4, 513], f16)
            nc.sync.dma_start(out=WlT, in_=WlT_d[:, :, :, :])
            bl = wout.tile([128, 2, 5], f32)
            nc.sync.dma_start(out=bl, in_=bl_d[:, :, :])

            opool = octx.enter_context(tc.tile_pool(name="opool", bufs=3))
            spool = octx.enter_context(tc.tile_pool(name="spool", bufs=2))
            opsum = octx.enter_context(tc.tile_pool(name="opsum", bufs=4, space="PSUM"))

            tc_chunk = min(16, t_total)
            assert t_total % tc_chunk == 0
            nchunks_total = t_total // tc_chunk
            if shard_output:
                assert nchunks_total % NCORES == 0
                nchunks = nchunks_total // NCORES
                pid = nc.sync.partition_id()
            else:
                nchunks = nchunks_total

            for c in range(nchunks):
                if shard_output:
                    tsl = bass.ds(pid * (nchunks * tc_chunk) + c * tc_chunk, tc_chunk)
                else:
                    tsl = slice(c * tc_chunk, (c + 1) * tc_chunk)
                rhs = []
                for kt in range(4):
                    rt = opool.tile([128, tc_chunk * 32], f16, tag=f"rhs{kt}")
                    nc.sync.dma_start(
                        out=rt,
                        in_=h3T[tsl, :, 32 * kt:32 * kt + 32]
                        .rearrange("t k b -> k t b"))
                    rhs.append(rt)
                for m in range(5):
                    fp = 128 if m < 4 else 1
                    xt = opool.tile([128, tc_chunk * 32], f16, tag="xchunk")
                    nc.sync.dma_start(
                        out=xt[:fp], in_=xo_d[m, 0:fp, tsl, :])
                    ss = []
                    for hd in range(2):
                        ps = opsum.tile([128, tc_chunk * 32], f32, tag=f"ops{hd}")
                        for kt in range(4):
                            nc.tensor.matmul(
                                ps[:fp], WlT[:, hd, kt, m * 128:m * 128 + fp],
                                rhs[kt], start=(kt == 0), stop=(kt == 3))
                        s = spool.tile([128, tc_chunk * 32], f32, tag=f"s{hd}")
                        nc.scalar.activation(
                            s[:fp], ps[:fp], AF.Relu, bias=bl[0:fp, hd, m:m + 1])
                        ss.append(s)
                    den = spool.tile([128, tc_chunk * 32], f32, tag="den")
                    nc.vector.tensor_add(den[:fp], ss[0][:fp], ss[1][:fp])
                    nc.vector.tensor_scalar_add(den[:fp], den[:fp], 1e-16)
                    rden = spool.tile([128, tc_chunk * 32], f32, tag="rden")
                    nc.vector.reciprocal(rden[:fp], den[:fp])
                    xr = spool.tile([128, tc_chunk * 32], f32, tag="xr")
                    nc.vector.tensor_mul(xr[:fp], xt[:fp], rden[:fp])
                    for hd, out_d in ((0, out1_d), (1, out2_d)):
                        o = spool.tile([128, tc_chunk * 32], f32, tag=f"o{hd}")
                        nc.vector.tensor_mul(o[:fp], ss[hd][:fp], xr[:fp])
                        nc.sync.dma_start(
                            out=out_d[m, 0:fp, tsl, :],
                            in_=o[:fp].rearrange("f (t b) -> f t b", b=32))

    nc.finalize()
    return nc


# ---------------------------------------------------------------------------
# Entry point
# ---------------------------------------------------------------------------

class _Runner:
    """Caches the compiled PJRT executable so repeat calls only pay
    dispatch + device execution (mirrors bass2jax.run_bass_via_pjrt)."""

    def __init__(self, nc, n_cores):
        import jax
        import concourse.mybir as mybir
        from concourse import bass2jax
        from concourse.bass2jax import (
            _bass_exec_p, install_neuronx_cc_hook, partition_id_tensor)
        from jax.experimental.shard_map import shard_map
        from jax.sharding import Mesh, PartitionSpec

        install_neuronx_cc_hook()
        self.jax = jax
        self.n_cores = n_cores
        partition_name = (nc.partition_id_tensor.name
                          if nc.partition_id_tensor else None)
        in_names, out_names, out_avals, zero_outs = [], [], [], []
        for alloc in nc.m.functions[0].allocations:
            if not isinstance(alloc, mybir.MemoryLocationSet):
                continue
            name = alloc.memorylocations[0].name
            if alloc.kind == "ExternalInput":
                if name != partition_name:
                    in_names.append(name)
            elif alloc.kind == "ExternalOutput":
                shape = tuple(alloc.tensor_shape)
                dtype = mybir.dt.np(alloc.dtype)
                out_names.append(name)
                out_avals.append(jax.core.ShapedArray(shape, dtype))
                zero_outs.append(np.zeros(shape, dtype))
        n_params = len(in_names)
        self.in_names = list(in_names)
        self.out_names = out_names
        self.out_avals = out_avals
        self.zero_outs = zero_outs
        all_in = in_names + out_names
        if partition_name is not None:
            all_in.append(partition_name)

        def _body(*args):
            operands = list(args)
            if partition_name is not None:
                operands.append(partition_id_tensor())
            return tuple(_bass_exec_p.bind(
                *operands, out_avals=tuple(out_avals), in_names=tuple(all_in),
                out_names=tuple(out_names), lowering_input_output_aliases=(),
                sim_require_finite=True, sim_require_nnan=True, nc=nc))

        devices = jax.devices()[:n_cores]
        self.mesh = Mesh(np.asarray(devices), ("core",))
        self.pspec = PartitionSpec("core")
        n_out = len(out_names)
        self.sharded = jax.jit(
            shard_map(_body, mesh=self.mesh,
                      in_specs=(self.pspec,) * (n_params + n_out),
                      out_specs=(self.pspec,) * n_out,
                      check_rep=False),
            keep_unused=True)

    def prepare(self, in_map):
        """Concat per-core inputs + zero out-buffers, device_put once."""
        import jax
        from jax.sharding import NamedSharding
        sh = NamedSharding(self.mesh, self.pspec)
        args = [np.concatenate([np.asarray(in_map[n])] * self.n_cores, axis=0)
                for n in self.in_names]
        args += [np.zeros((self.n_cores * z.shape[0], *z.shape[1:]), z.dtype)
                 for z in self.zero_outs]
        return [jax.device_put(a, sh) for a in args]

    def call(self, concat_in):
        return self.sharded(*concat_in)

    def results(self, outs, sharded_t):
        """Outputs as numpy. If sharded_t, stitch per-core time slices."""
        res = {}
        for i, name in enumerate(self.out_names):
            a = np.asarray(outs[i]).reshape(self.n_cores, *self.out_avals[i].shape)
            if sharded_t:
                tc = a.shape[3] // self.n_cores
                full = np.empty(self.out_avals[i].shape, a.dtype)
                for c in range(self.n_cores):
                    full[:, :, c * tc:(c + 1) * tc, :] = \
                        a[c][:, :, c * tc:(c + 1) * tc, :]
                res[name] = full
            else:
                res[name] = a[0]
        return res


def _shardable(t_steps):
    return t_steps % (16 * NCORES) == 0


def _get_runner(t_steps):
    key = t_steps
    if key not in _CACHE:
        nc = build_nc(t_steps, _shardable(t_steps))
        _CACHE[key] = _Runner(nc, NCORES)
    return _CACHE[key]


def _run(inputs, t_steps=T, time_reps=0):
    import time as _time
    r = _get_runner(t_steps)
    p = prep_inputs(inputs, t_steps)
    concat_in = r.prepare(p)
    outs = r.call(concat_in)  # first call compiles
    out = r.results(outs, _shardable(t_steps))
    o1 = _unpack_out(out["out1"], t_steps)
    o2 = _unpack_out(out["out2"], t_steps)

    times = []
    for _ in range(time_reps):
        t0 = _time.time()
        outs = r.call(concat_in)
        for o in outs:
            o.block_until_ready()
        times.append(_time.time() - t0)
    return (o1, o2), times


def _unpack_out(o, t_steps):
    """[5, 128, T, B] f-major -> [B, T, 513]."""
    return np.ascontiguousarray(
        np.transpose(o.reshape(640, t_steps, B)[:F], (2, 1, 0)))


def kernel(**inputs):
    (o1, o2), _ = _run(inputs, T)
    return (o1, o2)
